# revision 4
# baseline (speedup 1.0000x reference)
"""DMoN layer on 8 Trainium2 NeuronCores.

reference:
    h = relu(x @ W1 + b1); C = softmax(h @ W2 + b2)
    edge_sum = sum_e C[src_e] . C[dst_e]; loss = -edge_sum / E
    returns (C, loss)

Distribution: nodes sharded 8 ways for the MLP (each core computes its C
shard), AllGather of C across the cores, edges sharded 8 ways for the
gather/dot phase (indirect-DMA row gathers from the allgathered C), host sums
the 8 partial edge sums.
"""
import sys

sys.path.insert(0, "/opt/trn_rl_repo")
import numpy as np

import concourse.bass as bass
import concourse.mybir as mybir
import concourse.tile as tile
from concourse import bacc
from concourse.bass_utils import run_bass_kernel_spmd

NCORES = 8
N, D_IN, D_H, K, E = 100000, 128, 256, 16, 3200000
NS = 12500            # real nodes per core
NSP = 12544           # padded nodes per core (98 * 128)
NCHUNK = 98           # 128-node chunks per core
NP = NSP * NCORES     # padded global nodes
ES = E // NCORES      # edges per core
ECOLS = ES // 128     # 3125 index columns of 128
F = 125               # index columns per gather group
NGROUP = ECOLS // F   # 25 groups per core

_cache = {}


def _build():
    nc = bacc.Bacc("TRN2", target_bir_lowering=False, debug=False, num_devices=NCORES)
    dt = mybir.dt

    xT_in = nc.dram_tensor("xT_in", [128, NSP], dt.float32, kind="ExternalInput")
    w1_in = nc.dram_tensor("w1_in", [128, D_H], dt.float32, kind="ExternalInput")
    b1_in = nc.dram_tensor("b1_in", [128, 2], dt.float32, kind="ExternalInput")
    w2_in = nc.dram_tensor("w2_in", [128, 2 * K], dt.float32, kind="ExternalInput")
    b2_in = nc.dram_tensor("b2_in", [1, K], dt.float32, kind="ExternalInput")
    idx_in = nc.dram_tensor("idx_in", [128, 2 * ECOLS], dt.int32, kind="ExternalInput")

    c_out = nc.dram_tensor("c_out", [128, NCHUNK * K], dt.float32, kind="ExternalOutput")
    s_out = nc.dram_tensor("s_out", [128, 1], dt.float32, kind="ExternalOutput")

    bounce = nc.dram_tensor("bounce", [NSP, K], dt.float32, kind="Internal")
    c_full = nc.dram_tensor("c_full", [NP, K], dt.float32, kind="Internal", addr_space="Shared")

    with tile.TileContext(nc) as tc:
        with (
            tc.tile_pool(name="const", bufs=1) as cst,
            tc.tile_pool(name="xt", bufs=3) as xtp,
            tc.tile_pool(name="ps", bufs=2, space="PSUM") as psp,
            tc.tile_pool(name="psl", bufs=2, space="PSUM") as pslp,
            tc.tile_pool(name="sm", bufs=3) as smp,
            tc.tile_pool(name="gat", bufs=3) as gatp,
        ):
            # ---- constants / params
            w1t = cst.tile([128, D_H], dt.float32)
            nc.sync.dma_start(w1t[:], w1_in[:])
            b1t = cst.tile([128, 2], dt.float32)
            nc.sync.dma_start(b1t[:], b1_in[:])
            w2t = cst.tile([128, 2 * K], dt.float32)
            nc.sync.dma_start(w2t[:], w2_in[:])
            b2t = cst.tile([1, K], dt.float32)
            nc.sync.dma_start(b2t[:], b2_in[:])
            ones = cst.tile([1, 128], dt.float32)
            nc.vector.memset(ones[:], 1.0)

            # ---- edge index tiles: one big SWDGE load (same queue as gathers)
            idxt = cst.tile([128, 2 * ECOLS], dt.int32)
            nc.gpsimd.dma_start(idxt[:], idx_in[:])
            # dummy SWDGE read of idxt so the pool queue observes its tick
            scrap = cst.tile([1, 1], dt.int32)
            nc.gpsimd.dma_start(scrap[:], idxt[:1, :1])

            # ---- MLP: C shard, kept fully in SBUF
            csb = cst.tile([128, NCHUNK * K], dt.float32)
            negmax = smp.tile([128, 1], dt.float32, tag="negmax")
            sums = smp.tile([128, 1], dt.float32, tag="sums")
            rinv = smp.tile([128, 1], dt.float32, tag="rinv")

            node_tiles = [512] * 24 + [256]
            base = 0
            for nt in node_tiles:
                xt = xtp.tile([128, 512], dt.float32, tag="xt")
                nc.sync.dma_start(xt[:, :nt], xT_in[:, base:base + nt])
                hts = []
                for s in range(2):
                    psh = psp.tile([128, 512], dt.float32, tag=f"psh{s}")
                    nc.tensor.matmul(
                        out=psh[:, :nt],
                        lhsT=w1t[:, s * 128:(s + 1) * 128],
                        rhs=xt[:, :nt],
                        start=True, stop=True,
                    )
                    ht = xtp.tile([128, 512], dt.float32, tag=f"ht{s}")
                    nc.scalar.activation(
                        out=ht[:, :nt], in_=psh[:, :nt],
                        func=mybir.ActivationFunctionType.Relu,
                        bias=b1t[:, s:s + 1],
                    )
                    hts.append(ht)
                for c0 in range(0, nt, 128):
                    chunk = (base + c0) // 128
                    psl = pslp.tile([128, K], dt.float32, tag="psl")
                    nc.tensor.matmul(out=psl[:], lhsT=ones[:], rhs=b2t[:],
                                     start=True, stop=False)
                    nc.tensor.matmul(out=psl[:], lhsT=hts[0][:, c0:c0 + 128],
                                     rhs=w2t[:, 0:K], start=False, stop=False)
                    nc.tensor.matmul(out=psl[:], lhsT=hts[1][:, c0:c0 + 128],
                                     rhs=w2t[:, K:2 * K], start=False, stop=True)
                    nc.vector.tensor_reduce(
                        out=negmax[:], in_=psl[:], axis=mybir.AxisListType.X,
                        op=mybir.AluOpType.max, negate=True,
                    )
                    nc.scalar.activation(
                        out=csb[:, chunk * K:(chunk + 1) * K], in_=psl[:],
                        func=mybir.ActivationFunctionType.Exp,
                        bias=negmax[:], accum_out=sums[:],
                    )
                    nc.vector.reciprocal(rinv[:], sums[:])
                    nc.vector.tensor_scalar_mul(
                        csb[:, chunk * K:(chunk + 1) * K],
                        csb[:, chunk * K:(chunk + 1) * K], rinv[:],
                    )
                base += nt

            # ---- ship C shard: output + collective bounce (single fat DMAs)
            nc.sync.dma_start(c_out[:], csb[:])
            nc.gpsimd.dma_start(
                bounce[:].rearrange("(p c) k -> p (c k)", p=128), csb[:]
            )
            nc.gpsimd.collective_compute(
                "AllGather", mybir.AluOpType.bypass,
                replica_groups=[list(range(NCORES))],
                ins=[bounce[:]], outs=[c_full[:]],
            )

            # ---- edge phase: gather C rows for src/dst, multiply, reduce
            acc = cst.tile([128, NGROUP], dt.float32)
            prod = cst.tile([128, F * K], dt.float32)
            for g in range(NGROUP):
                gats = gatp.tile([128, F * K], dt.float32, tag="gs")
                gatd = gatp.tile([128, F * K], dt.float32, tag="gd")
                for f in range(F):
                    nc.gpsimd.indirect_dma_start(
                        out=gats[:, f * K:(f + 1) * K], out_offset=None,
                        in_=c_full[:],
                        in_offset=bass.IndirectOffsetOnAxis(
                            ap=idxt[:, g * F + f:g * F + f + 1], axis=0),
                    )
                for f in range(F):
                    nc.gpsimd.indirect_dma_start(
                        out=gatd[:, f * K:(f + 1) * K], out_offset=None,
                        in_=c_full[:],
                        in_offset=bass.IndirectOffsetOnAxis(
                            ap=idxt[:, ECOLS + g * F + f:ECOLS + g * F + f + 1], axis=0),
                    )
                nc.vector.tensor_tensor(
                    out=prod[:], in0=gats[:], in1=gatd[:],
                    op=mybir.AluOpType.mult,
                )
                nc.vector.reduce_sum(acc[:, g:g + 1], prod[:],
                                     axis=mybir.AxisListType.X)
            total = cst.tile([128, 1], dt.float32)
            nc.vector.reduce_sum(total[:], acc[:], axis=mybir.AxisListType.X)
            nc.sync.dma_start(s_out[:], total[:])
    nc.finalize()
    return nc


def _node_perm_row(node):
    """Global padded-C row for global node id (vectorized)."""
    core = node // NS
    local = node - core * NS
    c = local // 128
    p = local - c * 128
    return core * NSP + p * NCHUNK + c


def kernel(x, edge_index, W1, b1, W2, b2):
    x = np.asarray(x, dtype=np.float32)
    edge_index = np.asarray(edge_index)
    W1 = np.asarray(W1, dtype=np.float32)
    b1 = np.asarray(b1, dtype=np.float32)
    W2 = np.asarray(W2, dtype=np.float32)
    b2 = np.asarray(b2, dtype=np.float32)

    # host-side sharding prep
    xT = np.ascontiguousarray(x.T)                      # [128, N]
    xTp = np.zeros((128, NP // NCORES * NCORES), np.float32)  # not used; per-shard pad below
    b1r = b1.reshape(2, 128).T.copy()                   # [128, 2]: col s = b1[s*128 + p]
    w2r = np.concatenate([W2[:128], W2[128:]], axis=1)  # [128, 2K]: cols [sK:(s+1)K] = W2[s*128+p]
    b2r = b2.reshape(1, K).copy()

    rows = _node_perm_row(edge_index.astype(np.int64))  # [2, E]
    rows = rows.astype(np.int32)

    in_maps = []
    for c in range(NCORES):
        xs = np.zeros((128, NSP), np.float32)
        xs[:, :NS] = xT[:, c * NS:(c + 1) * NS]
        src = rows[0, c * ES:(c + 1) * ES].reshape(128, ECOLS)
        dst = rows[1, c * ES:(c + 1) * ES].reshape(128, ECOLS)
        idx = np.concatenate([src, dst], axis=1)        # [128, 2*ECOLS]
        in_maps.append({
            "xT_in": np.ascontiguousarray(xs),
            "w1_in": W1,
            "b1_in": b1r,
            "w2_in": w2r,
            "b2_in": b2r,
            "idx_in": np.ascontiguousarray(idx),
        })

    if "nc" not in _cache:
        _cache["nc"] = _build()
    res = run_bass_kernel_spmd(_cache["nc"], in_maps, core_ids=list(range(NCORES)))

    C = np.empty((N, K), np.float32)
    edge_sum = 0.0
    for c in range(NCORES):
        cperm = res.results[c]["c_out"].reshape(128, NCHUNK, K)
        cshard = cperm.transpose(1, 0, 2).reshape(NSP, K)[:NS]
        C[c * NS:(c + 1) * NS] = cshard
        edge_sum += float(res.results[c]["s_out"].astype(np.float64).sum())

    loss = np.float32(-edge_sum / float(E))
    return C, loss
